# revision 44
# baseline (speedup 1.0000x reference)
"""Additive attention (B=4, Q=KV=512, H=256) on 8 Trainium2 NeuronCores.

Math (per batch b):
  q = queries @ W_q            (Q, H)
  k = keys    @ W_k            (KV, H)
  scores[i,j] = sum_h w_v[h] * tanh(q[i,h] + k[j,h])
  attn = softmax_j(scores masked to j < valid_lens[b])
  out  = attn @ values         (Q, V)

Separable expansion (P=10 pairs of sine-product features, fitted):
  tanh(q+k) ~ sum_p [alpha_p * Qf_p(q) + beta_p] * Kf_p(k)
so the h-reduction becomes a PE matmul with contraction (pair, h).

v2 schedule (same numerics as v1, restructured for the engines):
 - inputs land via one straight [128, X] DMA per tensor on the two HWDGE
   queues (sync: wk,kT; scalar: wq,qT,vals); host pre-packs partition-major.
 - PE warm-up matmuls at kernel start overlap the input DMA and flip the
   HAM clock gate to 2.4 GHz before the real matmuls arrive.
 - k/q projections write merged PSUM tiles so each sine pass is ONE ACT
   instruction (2-level AP over the two h-chunks).
 - beta_p * |wv_h| rides as a 257th column of each QC block, accumulating
   the exp-bias inside the same PSUM group as the scores (no extra PE work).
 - feature chain: all elementwise work on DVE (GpSimd elementwise poisons
   DVE ~4x via the shared SBUF port lock) except early-pair QC scaling and
   one output scale, which ride ACT's idle windows.  Dep-free filler
   matmuls keep PE busy while the first score group trickles in behind the
   QC blocks, so the HAM clock gate never re-throttles mid-kernel.
   Values/row-sum matmuls interleave with the next jc's score group so PE
   never waits on the exp.

Sharding: batch b -> cores {2b, 2b+1}, 256 query rows each.  Key windows
padded to JW = ceil(max valid /8)*8; padded columns masked (-1e6), padded
value rows zeroed.
"""

import sys
import types

import numpy as np

NEG = -1.0e6
NCORES = 8
NWARM = 7  # PE warm-up matmuls (N=512) overlapping the input DMA
TRACE = False  # test.py flips this to get a profiled run
LAST_RESULT = None  # BassKernelResults stash for test.py

# --- fitted expansion: tanh(x+y) ~ sum_p (alpha_p Qf_p(x) + beta_p) Kf_p(y)
# feature slots (same chain both sides):
#   sh=sin(.5wx) s1=sin(wx) s2=sin(2wx) c1d=sh^2 c2d=s1^2 c2t=c2d-.5
#   c4d2=c2t^2 s4t=s2*c2t c4t=c4d2-.125 s8t=s4t*c4t c8d=c4t^2
#   mixQ: c2s4=c2t*s4t s2c4=s2*c4t   mixK: c2c4=c2t*c4t s2s4=s2*s4t
FIT_W = 0.36
PAIRS = [  # (q_feature, k_feature)
    ("s1", "c1d"), ("c1d", "s1"), ("s2", "c2t"), ("c2t", "s2"),
    ("s4t", "c4t"), ("c4t", "s4t"), ("s8t", "c8d"), ("c8d", "s8t"),
    ("c2s4", "c2c4"), ("s2c4", "s2s4"),
]
ALPHAS = [-0.439680893, -4.32525681, -0.845301755, 0.0239388354,
          -5.14401459, -7.71941257, -175.582672, -319.866805,
          -10.9101526, 2.72429164]
BETAS = [0.0, 1.1382438, 0.0, 0.195903978, 0.0, 0.0444884607,
         0.0, 2.46151355, 0.0, 0.0]


def _install_axon_profile_hook():
    """antenv.axon_hooks is missing from this image; concourse needs it for
    trace=True under axon. Register the ctypes-based NTFF hook manually."""
    import antenv

    if "antenv.axon_hooks" in sys.modules:
        return
    m = types.ModuleType("antenv.axon_hooks")
    m._hook = None

    def _set(h):
        m._hook = h

    def _get():
        return m._hook

    m.set_axon_ntff_profile_hook = _set
    m.get_axon_ntff_profile_hook = _get
    sys.modules["antenv.axon_hooks"] = m
    antenv.axon_hooks = m
    try:
        from trn_agent_boot.trn_boot import _ntff_profile_via_ctypes

        m.set_axon_ntff_profile_hook(
            _ntff_profile_via_ctypes("/opt/axon/libaxon_pjrt.so")
        )
    except Exception:
        pass


def _patch_tile_drain():
    """The walrus build in this image allows at most ONE sync-wait command
    per instruction; Tile's kernel-tail drain carries every vector-clock
    wait on a single drain. Split them across a chain of drains."""
    import concourse.mybir as mybir
    import concourse.tile as tile
    from concourse.vector_clock import ScopedClock

    if getattr(tile.TileContext, "_drain_patched", False):
        return

    def _drain_and_barrier_chunked(self, tick_clock, wait_clock):
        d0 = self.nc.sync.drain()
        wait_clock.add_sem_waits(d0.ins, ScopedClock({None: tick_clock.global_clock}))
        si = d0.ins.sync_info
        waits = list(si.on_wait) if si is not None else []
        if len(waits) > 1:
            engs = [
                mybir.EngineType.SP,
                mybir.EngineType.DVE,
                mybir.EngineType.Activation,
                mybir.EngineType.PE,
                mybir.EngineType.Pool,
            ]
            d0.ins.sync_info = mybir.SyncInfo(
                on_wait=waits[:1], on_update=list(si.on_update)
            )
            for i in range(1, len(waits)):
                ev = mybir.InstEventSemaphore(
                    name=f"tail-wait-{i}",
                    engine=engs[i % len(engs)],
                    ins=[],
                    outs=[],
                    sync_info=mybir.SyncInfo(on_wait=[waits[i]], on_update=[]),
                )
                self.nc.register_instruction(ev)
                self.nc.cur_bb.bb.add_instruction(ev)

        self.nc.all_engine_barrier()
        assert self.sems is not None
        popped = self.nc._tile_sem_poison_stack.pop()
        assert popped is self._sem_poison
        self.nc.clear_and_free_semaphores(list(self.sems.allocated().values()))
        self.nc.all_engine_barrier()

    tile.TileContext._drain_and_barrier = _drain_and_barrier_chunked
    tile.TileContext._drain_patched = True


def _split_multi_waits(nc):
    """walrus here allows one sync-wait command per instruction; move extra
    waits onto standalone EventSemaphore instructions."""
    import concourse.mybir as mybir

    n = 0
    for fn in nc.m.functions:
        for blk in fn.blocks:
            out = []
            for inst in blk.instructions:
                si = inst.sync_info
                waits = list(si.on_wait) if si is not None else []
                if len(waits) > 1:
                    for k, w in enumerate(waits[:-1]):
                        ev = mybir.InstEventSemaphore(
                            name=f"{inst.name}-xw{k}",
                            engine=inst.engine,
                            ins=[],
                            outs=[],
                            sync_info=mybir.SyncInfo(on_wait=[w], on_update=[]),
                        )
                        out.append(ev)
                        n += 1
                    inst.sync_info = mybir.SyncInfo(
                        on_wait=[waits[-1]], on_update=list(si.on_update)
                    )
                out.append(inst)
            blk.instructions = out
    return n


def _ceil_to(x, m):
    return -(-int(x) // m) * m


# feature slot order in the per-side feature tile (each slot = [K | Q]);
# chosen so every batched pair-product has affine operand APs.
SLOTS = {"sh": 0, "s1": 1, "c1d": 2, "c2d": 3, "c2t": 4, "s2": 5,
         "c4d2": 6, "s4t": 7, "c4t": 8, "s8t": 9, "c8d": 10,
         "m1": 11, "m2": 12}
NSLOT = 13
QSLOT_OF = {"c2s4": "m1", "s2c4": "m2"}
KSLOT_OF = {"c2c4": "m1", "s2s4": "m2"}
# NOTE (measured): concurrent GpSimd elementwise ops and DVE ops slow each
# other ~4x via the shared SBUF port lock — GpSimd gets NO elementwise work.


def _build_program(D, V, H, JW, ROWS, NCH, lns, NFULL, LP, split_waits=True):
    """Uniform SPMD program: one batch per core, ROWS query rows, key
    window JW (padded; mask handles validity)."""
    import contextlib

    import concourse.bass as bass
    import concourse.mybir as mybir
    import concourse.tile as tile

    f32 = mybir.dt.float32
    f16 = mybir.dt.float16
    AF = mybir.ActivationFunctionType

    DC = D // 128
    HC = H // 128
    P = len(PAIRS)
    RC = ROWS // 128
    W = FIT_W
    BW = 258  # QC block width: 256 q-cols + 2 beta cols (even => 4B-aligned
    #           blocks keep the DVE tensor_scalar ops in 4x perf mode)

    nc = bass.Bass("TRN2", target_bir_lowering=False)
    d_wk = nc.declare_dram_parameter("wk", [128, DC * H], f16, isOutput=False)
    d_kT = nc.declare_dram_parameter("kT", [128, DC * JW], f16, isOutput=False)
    d_wq = nc.declare_dram_parameter("wq", [128, DC * H], f16, isOutput=False)
    d_qT = nc.declare_dram_parameter("qT", [128, DC * ROWS], f16, isOutput=False)
    if NFULL:
        d_valsf = nc.declare_dram_parameter("valsf", [128, NFULL * V], f16,
                                            isOutput=False)
    if LP:
        d_vals2 = nc.declare_dram_parameter("vals2", [128, V], f16,
                                            isOutput=False)
    # smalls: maskT (NCH cols) | qcs (P*HC cols) f32, wvb f16
    d_sm32 = nc.declare_dram_parameter("sm32", [128, NCH + P * HC], f32,
                                       isOutput=False)
    d_wvb = nc.declare_dram_parameter("wvb", [128, P * HC], f16, isOutput=False)
    d_out = nc.declare_dram_parameter("out", [ROWS, V], f16, isOutput=True)

    KW = HC * JW
    QW = HC * ROWS
    SW = KW + QW

    with tile.TileContext(nc) as tc:
        ctx = contextlib.ExitStack()
        with ctx:
            const_pool = ctx.enter_context(tc.tile_pool(name="const", bufs=1))
            in_pool = ctx.enter_context(tc.tile_pool(name="in", bufs=1))
            feat_pool = ctx.enter_context(tc.tile_pool(name="feat", bufs=1))
            qc_pool = ctx.enter_context(tc.tile_pool(name="qc", bufs=1))
            soft_pool = ctx.enter_context(tc.tile_pool(name="soft", bufs=1))
            out_pool = ctx.enter_context(tc.tile_pool(name="outp", bufs=1))
            ppsum = ctx.enter_context(tc.tile_pool(name="pp", bufs=1, space="PSUM"))
            scpsum = ctx.enter_context(tc.tile_pool(name="scp", bufs=2, space="PSUM"))
            opsum = ctx.enter_context(tc.tile_pool(name="op", bufs=2, space="PSUM"))
            smpsum = ctx.enter_context(tc.tile_pool(name="smp", bufs=1, space="PSUM"))

            warm = const_pool.tile([1, 2], f32)
            ones_f16 = const_pool.tile([128, 1], f16)
            wtile = const_pool.tile([128, 512], f16, name="wtile")
            sm32_sb = const_pool.tile([128, NCH + P * HC], f32, name="sm32")
            wvb_sb = const_pool.tile([128, P * HC], f16, name="wvb")
            maskT = sm32_sb[:, 0:NCH]
            qcs = sm32_sb[:, NCH:NCH + P * HC]

            wk_sb = in_pool.tile([128, DC * H], f16, name="wk")
            kT_sb = in_pool.tile([128, DC * JW], f16, name="kT")
            wq_sb = in_pool.tile([128, DC * H], f16, name="wq")
            qT_sb = in_pool.tile([128, DC * ROWS], f16, name="qT")
            if NFULL:
                valsf_sb = in_pool.tile([128, NFULL * V], f16, name="valsf")
            if LP:
                vals2_sb = in_pool.tile([128, V], f16, name="vals2")

            F = feat_pool.tile([128, NSLOT * SW], f16, name="F")
            QC = qc_pool.tile([128, P * HC * BW], f16, name="QC")
            eT = soft_pool.tile([128, NCH * ROWS], f16, name="eT")
            biasc = soft_pool.tile([128, NCH], f32, name="biasc")
            rinv = soft_pool.tile([128, RC], f32, name="rinv")
            out_sb = out_pool.tile([128, RC * V], f16, name="osb")

            pk = ppsum.tile([128, HC * 512], f32, tag="pk", name="pk")
            pq = ppsum.tile([128, 512], f32, tag="pq", name="pq")
            misc = smpsum.tile([128, 512], f32, tag="misc", name="misc")

            # ---- memsets (DVE) then DMA triggers, one per tensor
            nc.vector.memset(wtile[:], 0.25)
            nc.vector.memset(warm[:], 0.5)
            nc.vector.memset(ones_f16[:], 1.0)
            # k-projection inputs split across BOTH HWDGE queues so they
            # stream concurrently; q inputs next, values last.
            nc.sync.dma_start(out=wk_sb[:], in_=d_wk[:])
            # kT in two halves so the first k-proj matmuls start earlier
            kh = (DC // 2) * JW
            nc.scalar.dma_start(out=kT_sb[:, 0:kh], in_=d_kT[:, 0:kh])
            nc.scalar.dma_start(out=kT_sb[:, kh:], in_=d_kT[:, kh:])
            nc.sync.dma_start(out=qT_sb[:], in_=d_qT[:])
            # Sin table load rides between the scalar queue's triggers
            nc.scalar.activation(warm[0:1, 0:1], warm[0:1, 1:2], AF.Sin)
            nc.scalar.dma_start(out=wq_sb[:], in_=d_wq[:])
            if NFULL:
                nc.scalar.dma_start(out=valsf_sb[:], in_=d_valsf[:])
            if LP:
                nc.sync.dma_start(out=vals2_sb[:LP, :], in_=d_vals2[:LP, :])
            nc.gpsimd.dma_start(out=sm32_sb[:], in_=d_sm32[:])
            nc.gpsimd.dma_start(out=wvb_sb[:], in_=d_wvb[:])

            # beta columns -> cols 256,257 of each QC block (the pad col gets
            # the same value so nothing in the block is uninitialized; psum
            # col 257 is simply never read)
            qcv = QC[:]
            wvb_src = wvb_sb[:, 0:P * HC].rearrange("p (b c) -> p b c", c=1)
            for pad in (256, 257):
                wvb_dst = bass.AP(qcv.tensor, qcv.offset + pad,
                                  [qcv.ap[0], [BW, P * HC], [1, 1]])
                nc.vector.tensor_copy(wvb_dst, wvb_src)

            # ---- PE warm-up: keep PE busy through the DMA so HAM un-throttles
            for _ in range(NWARM):
                nc.tensor.matmul(pq[:, 0:512], wtile[:, 0:128],
                                 wtile[:, 0:512], start=True, stop=True)

            # ---- projections (PE); pk h-chunks bank-aligned at hc*512
            for hc in range(HC):
                for dc in range(DC):
                    nc.tensor.matmul(
                        pk[:, hc * 512:hc * 512 + JW],
                        wk_sb[:, dc * H + hc * 128:dc * H + (hc + 1) * 128],
                        kT_sb[:, dc * JW:(dc + 1) * JW],
                        start=(dc == 0), stop=(dc == DC - 1),
                    )
            for hc in range(HC):
                for dc in range(DC):
                    nc.tensor.matmul(
                        pq[:, hc * ROWS:(hc + 1) * ROWS],
                        wq_sb[:, dc * H + hc * 128:dc * H + (hc + 1) * 128],
                        qT_sb[:, dc * ROWS:(dc + 1) * ROWS],
                        start=(dc == 0), stop=(dc == DC - 1),
                    )

            # ---- feature slot helpers (slot s = [K (KW) | Q (QW)])
            def kslot(s):
                o = SLOTS[s] * SW
                return F[:, o:o + KW]

            def qslot(s):
                o = SLOTS[s] * SW + KW
                return F[:, o:o + QW]

            # ---- sines: ONE ACT instr per side per pass
            pkap = pk[:]
            pk_src = bass.AP(pkap.tensor, pkap.offset,
                             [pkap.ap[0], [512, HC], [1, JW]])
            # all K sines first: the K chain (score stationaries) is the
            # longest dependency spine, so unblock it earliest
            for scale, s in ((0.5 * W, "sh"), (W, "s1"), (2.0 * W, "s2")):
                nc.scalar.activation(
                    kslot(s).rearrange("p (c j) -> p c j", j=JW),
                    pk_src, AF.Sin, scale=scale)
            for scale, s in ((0.5 * W, "sh"), (W, "s1"), (2.0 * W, "s2")):
                nc.scalar.activation(qslot(s), pq[:, 0:QW], AF.Sin, scale=scale)
            # second PE warm-up block: covers the sines/chain window (PE
            # would otherwise idle >3.4us and HAM would re-throttle); targets
            # the misc bank, which has NO readers yet, so it runs dep-free
            for _ in range(8):
                nc.tensor.matmul(misc[:, 0:512], wtile[:, 0:128],
                                 wtile[:, 0:512], start=True, stop=True)

            # ---- DVE product chains, K and Q sides fused per instruction
            fb = F[:]

            def kp_(sa, sb):
                return bass.AP(fb.tensor, fb.offset + SLOTS[sa] * SW,
                               [fb.ap[0], [(SLOTS[sb] - SLOTS[sa]) * SW, 2],
                                [1, KW]])

            def qp_(sa, sb):
                return bass.AP(fb.tensor, fb.offset + SLOTS[sa] * SW + KW,
                               [fb.ap[0], [(SLOTS[sb] - SLOTS[sa]) * SW, 2],
                                [1, QW]])

            def qc_block(p, hc):
                blk = p * HC + hc
                return QC[:, blk * BW:blk * BW + 256]

            def qc_scale(p, act=False):
                qs2 = QSLOT_OF.get(PAIRS[p][0], PAIRS[p][0])
                for hc in range(HC):
                    o = SLOTS[qs2] * SW + KW + hc * ROWS
                    if act:
                        nc.scalar.activation(
                            qc_block(p, hc), F[:, o:o + ROWS], AF.Copy,
                            scale=qcs[:, p * HC + hc:p * HC + hc + 1])
                    else:
                        nc.vector.tensor_scalar_mul(
                            qc_block(p, hc), F[:, o:o + ROWS],
                            qcs[:, p * HC + hc:p * HC + hc + 1])

            # Everything elementwise on DVE (GpSimd would poison it via the
            # shared SBUF port); pairs 6/7 QC ride ACT's idle window.
            # K chain first (its features are the score stationaries):
            nc.vector.tensor_mul(kp_("c1d", "c2d"), kp_("sh", "s1"),
                                 kp_("sh", "s1"))
            nc.vector.tensor_scalar_sub(kslot("c2t"), kslot("c2d"), 0.5)
            nc.vector.tensor_mul(kp_("c4d2", "s4t"), kp_("c2t", "c2t"),
                                 kp_("c2t", "s2"))
            nc.vector.tensor_scalar_sub(kslot("c4t"), kslot("c4d2"), 0.125)
            nc.vector.tensor_mul(kp_("s8t", "c8d"), kp_("s4t", "c4t"),
                                 kp_("c4t", "c4t"))
            # early-pair QC rides ACT (idle after the sines; p0/p2 read
            # ACT's own sine outputs); DVE keeps the chain spine + late QC
            qc_scale(0, act=True)   # q-feat s1
            qc_scale(2, act=True)   # q-feat s2
            # both K mix slots in ONE fused op: (m1,m2) = (c2t,s2)*(c4t,s4t);
            # the second source pair steps backwards (c4t@8 -> s4t@7)
            nc.vector.tensor_mul(kp_("m1", "m2"), kp_("c2t", "s2"),
                                 kp_("c4t", "s4t"))
            # K feature slots complete here.
            # Q chain with per-pair QC as each feature lands:
            nc.vector.tensor_mul(qp_("c1d", "c2d"), qp_("sh", "s1"),
                                 qp_("sh", "s1"))
            qc_scale(1, act=True)
            nc.vector.tensor_scalar_sub(qslot("c2t"), qslot("c2d"), 0.5)
            qc_scale(3, act=True)
            nc.vector.tensor_mul(qp_("c4d2", "s4t"), qp_("c2t", "c2t"),
                                 qp_("c2t", "s2"))
            qc_scale(4, act=True)
            # Exp table load after ACT's QC block (data-dep blocks hoisting)
            nc.scalar.activation(warm[0:1, 0:1], qslot("s2")[0:1, 0:1], AF.Exp)
            nc.vector.tensor_scalar_sub(qslot("c4t"), qslot("c4d2"), 0.125)
            qc_scale(5)
            nc.vector.tensor_mul(qp_("s8t", "c8d"), qp_("s4t", "c4t"),
                                 qp_("c4t", "c4t"))
            qc_scale(6)
            nc.vector.tensor_mul(qp_("m1", "m2"), qp_("c2t", "s2"),
                                 qp_("s4t", "c4t"))
            qc_scale(7)
            qc_scale(8)
            qc_scale(9)

            # ---- scores scT[j,i], PAIR-major: each QC block arrival unlocks
            # all NCH jc-chunks' matmuls at once, so PE stays busy (and the
            # HAM clock gate stays open) while the QC blocks trickle in.
            pouts = [opsum.tile([128, V], f32, tag="po", name=f"pout{rc}")
                     for rc in range(RC)]

            def vals_view(jc, ln):
                if jc < NFULL:
                    return valsf_sb[:ln, jc * V:(jc + 1) * V]
                return vals2_sb[:ln, :]

            def score_mm(psc, p, jc, hc, start, stop):
                ln = lns[jc]
                ks = KSLOT_OF.get(PAIRS[p][1], PAIRS[p][1])
                nc.tensor.matmul(
                    psc[:ln, 0:BW],
                    F[:, SLOTS[ks] * SW + hc * JW + jc * 128:
                       SLOTS[ks] * SW + hc * JW + jc * 128 + ln],
                    QC[:, (p * HC + hc) * BW:(p * HC + hc + 1) * BW],
                    start=start, stop=stop,
                )

            def emit_softmax(jc):
                ln = lns[jc]
                psc = pscs[jc]
                nc.vector.tensor_add(biasc[:ln, jc:jc + 1],
                                     maskT[:ln, jc:jc + 1],
                                     psc[:ln, 256:257])
                nc.scalar.activation(
                    eT[:ln, jc * ROWS:(jc + 1) * ROWS],
                    psc[:ln, 0:ROWS], AF.Exp,
                    bias=biasc[:ln, jc:jc + 1],
                )

            def emit_values(jc):
                ln = lns[jc]
                for rc in range(RC):
                    eblk = eT[:ln, jc * ROWS + rc * 128:jc * ROWS + rc * 128 + 128]
                    # groups interleave across jc, so each accumulator needs
                    # its own PSUM bank: rc0 sums -> misc, rc1 sums -> pq;
                    # pout rc0 -> opsum, pout rc1 -> a pk bank (dead now)
                    rsum_dst = misc[:, 0:1] if rc == 0 else pq[:, 0:1]
                    pout_dst = pouts[rc][:]
                    nc.tensor.matmul(
                        rsum_dst, eblk, ones_f16[:ln, 0:1],
                        start=(jc == 0), stop=(jc == NCH - 1))
                    nc.tensor.matmul(
                        pout_dst, eblk, vals_view(jc, ln),
                        start=(jc == 0), stop=(jc == NCH - 1))

            # jc-major: consecutive matmuls accumulate into the SAME psum
            # bank, which keeps the PE back-to-back pipeline (pair-major
            # bank-alternation measured ~3x slower per matmul)
            pscs = []
            for jc in range(NCH):
                psc = scpsum.tile([128, 512], f32, tag="sc",
                                  name=f"psc{jc}")
                pscs.append(psc)
                nmm = 0
                for p in range(P):
                    for hc in range(HC):
                        score_mm(psc, p, jc, hc, start=(nmm == 0),
                                 stop=(nmm == P * HC - 1))
                        nmm += 1
                    if jc == 0 and 1 <= p <= 6:
                        # dep-free filler matmuls: the first score group is
                        # paced by the QC trickle (~30% PE duty), which lets
                        # the HAM clock gate re-throttle; keep the PE busy.
                        # Pairs 1-3 wait on ACT's slower QC cadence and
                        # showed residual ~1us gaps -> one extra filler.
                        for _ in range(3 if p <= 3 else 2):
                            nc.tensor.matmul(misc[:, 0:256], wtile[:, 0:128],
                                             wtile[:, 0:256],
                                             start=True, stop=True)
                emit_softmax(jc)
                if jc > 0:
                    emit_values(jc - 1)
            emit_values(NCH - 1)

            # ---- normalize + store
            nc.vector.reciprocal(rinv[:, 0:1], misc[:, 0:1])
            nc.vector.reciprocal(rinv[:, 1:2], pq[:, 0:1])
            for rc in range(RC):
                pout_src = pouts[rc][:]
                # split the two output scales across ACT and DVE
                if rc == 0:
                    nc.scalar.activation(
                        out_sb[:, rc * V:(rc + 1) * V], pout_src,
                        AF.Copy, scale=rinv[:, rc:rc + 1])
                else:
                    nc.vector.tensor_scalar_mul(
                        out_sb[:, rc * V:(rc + 1) * V], pout_src,
                        rinv[:, rc:rc + 1])
                nc.sync.dma_start(
                    out=d_out[rc * 128:(rc + 1) * 128, :],
                    in_=out_sb[:, rc * V:(rc + 1) * V])

    if split_waits:
        _split_multi_waits(nc)
    return nc


def _pack_pm(a, cols):
    """[D, cols] -> partition-major [128, (D//128)*cols] fp16."""
    D = a.shape[0]
    dc = D // 128
    return np.ascontiguousarray(
        a.reshape(dc, 128, cols).transpose(1, 0, 2).reshape(128, dc * cols)
        .astype(np.float16))


def kernel(queries, keys, values, valid_lens, W_q, W_k, w_v):
    global LAST_RESULT
    _install_axon_profile_hook()
    _patch_tile_drain()
    from concourse.bass_utils import run_bass_kernel_spmd

    f16 = np.float16
    queries = np.ascontiguousarray(queries, dtype=np.float32)
    keys = np.ascontiguousarray(keys, dtype=np.float32)
    values = np.ascontiguousarray(values, dtype=np.float32)
    W_q = np.ascontiguousarray(W_q, dtype=np.float32)
    W_k = np.ascontiguousarray(W_k, dtype=np.float32)
    w_v = np.ascontiguousarray(w_v, dtype=np.float32)
    vl = np.asarray(valid_lens).astype(np.int64)

    B, Q, D = queries.shape
    KV = keys.shape[1]
    V = values.shape[2]
    H = W_q.shape[1]
    P = len(PAIRS)
    CPB = NCORES // B          # cores per batch
    ROWS = Q // CPB            # query rows per core
    HC = H // 128

    jms = [min(KV, int(v)) for v in vl]
    JW = min(KV, _ceil_to(max(jms), 8))
    NCH = _ceil_to(JW, 128) // 128
    lns = [min(128, JW - jc * 128) for jc in range(NCH)]
    LP = 0 if JW % 128 == 0 else JW - (NCH - 1) * 128
    NFULL = NCH if LP == 0 else NCH - 1

    # fold sign(w_v) into the projections (tanh is odd)
    sgn = np.where(w_v >= 0, 1.0, -1.0).astype(np.float32)
    wva = np.abs(w_v)
    Wq_f = _pack_pm(W_q * sgn[None, :], H)
    Wk_f = _pack_pm(W_k * sgn[None, :], H)

    nc = _build_program(D, V, H, JW, ROWS, NCH, lns, NFULL, LP)

    qcs_cols = [ALPHAS[p] * wva[hc * 128:(hc + 1) * 128]
                for p in range(P) for hc in range(HC)]
    wvb_cols = [BETAS[p] * wva[hc * 128:(hc + 1) * 128]
                for p in range(P) for hc in range(HC)]
    qcs = np.stack(qcs_cols, axis=1).astype(np.float32)
    wvb = np.ascontiguousarray(np.stack(wvb_cols, axis=1).astype(f16))

    in_maps = []
    for c in range(NCORES):
        b = c // CPB
        rh = c % CPB
        jm = jms[b]
        qT = _pack_pm(
            np.ascontiguousarray(queries[b, rh * ROWS:(rh + 1) * ROWS, :].T),
            ROWS)
        kTp = np.zeros((D, JW), np.float32)
        kTp[:, :jm] = keys[b, :jm, :].T
        kT = _pack_pm(kTp, JW)
        j = np.arange(128)
        maskT = np.stack(
            [np.where(jc * 128 + j < jm, 0.0, NEG).astype(np.float32)
             for jc in range(NCH)], axis=1)
        sm32 = np.ascontiguousarray(
            np.concatenate([maskT, qcs], axis=1).astype(np.float32))
        im = {"wk": Wk_f, "kT": kT, "wq": Wq_f, "qT": qT,
              "sm32": sm32, "wvb": wvb}
        if NFULL:
            vf = np.zeros((128, NFULL * V), f16)
            for jc in range(NFULL):
                nrows = max(0, min(128, jm - jc * 128))
                if nrows:
                    vf[0:nrows, jc * V:(jc + 1) * V] = \
                        values[b, jc * 128:jc * 128 + nrows, :].astype(f16)
            im["valsf"] = np.ascontiguousarray(vf)
        if LP:
            v2 = np.zeros((128, V), f16)
            nrows = max(0, min(LP, jm - NFULL * 128))
            if nrows:
                v2[0:nrows] = values[b, NFULL * 128:NFULL * 128 + nrows, :] \
                    .astype(f16)
            im["vals2"] = v2
        in_maps.append(im)

    res = run_bass_kernel_spmd(
        nc, in_maps, core_ids=list(range(NCORES)), trace=TRACE
    )
    LAST_RESULT = res

    out = np.empty((B, Q, V), np.float32)
    for c in range(NCORES):
        b = c // CPB
        rh = c % CPB
        out[b, rh * ROWS:(rh + 1) * ROWS, :] = res.results[c]["out"].astype(
            np.float32)
    return out


# revision 46
# speedup vs baseline: 1.1250x; 1.1250x over previous
"""Additive attention (B=4, Q=KV=512, H=256) on 8 Trainium2 NeuronCores.

Math (per batch b):
  q = queries @ W_q            (Q, H)
  k = keys    @ W_k            (KV, H)
  scores[i,j] = sum_h w_v[h] * tanh(q[i,h] + k[j,h])
  attn = softmax_j(scores masked to j < valid_lens[b])
  out  = attn @ values         (Q, V)

Separable expansion (P=10 pairs of sine-product features, fitted):
  tanh(q+k) ~ sum_p [alpha_p * Qf_p(q) + beta_p] * Kf_p(k)
so the h-reduction becomes a PE matmul with contraction (pair, h).

v2 schedule (same numerics as v1, restructured for the engines):
 - inputs land via one straight [128, X] DMA per tensor on the two HWDGE
   queues (sync: wk,kT; scalar: wq,qT,vals); host pre-packs partition-major.
 - PE warm-up matmuls at kernel start overlap the input DMA and flip the
   HAM clock gate to 2.4 GHz before the real matmuls arrive.
 - k/q projections write merged PSUM tiles so each sine pass is ONE ACT
   instruction (2-level AP over the two h-chunks).
 - beta_p * |wv_h| rides as a 257th column of each QC block, accumulating
   the exp-bias inside the same PSUM group as the scores (no extra PE work).
 - feature chain: all elementwise work on DVE (GpSimd elementwise poisons
   DVE ~4x via the shared SBUF port lock) except early-pair QC scaling and
   one output scale, which ride ACT's idle windows.  Dep-free filler
   matmuls keep PE busy while the first score group trickles in behind the
   QC blocks, so the HAM clock gate never re-throttles mid-kernel.
   Values/row-sum matmuls interleave with the next jc's score group so PE
   never waits on the exp.

Sharding: batch b -> cores {2b, 2b+1}, 256 query rows each.  Key windows
padded to JW = ceil(max valid /8)*8; padded columns masked (-1e6), padded
value rows zeroed.
"""

import sys
import types

import numpy as np

NEG = -1.0e6
NCORES = 8
NWARM = 7  # PE warm-up matmuls (N=512) overlapping the input DMA
TRACE = False  # test.py flips this to get a profiled run
LAST_RESULT = None  # BassKernelResults stash for test.py

# --- fitted expansion: tanh(x+y) ~ sum_p (alpha_p Qf_p(x) + beta_p) Kf_p(y)
# feature slots (same chain both sides):
#   sh=sin(.5wx) s1=sin(wx) s2=sin(2wx) c1d=sh^2 c2d=s1^2 c2t=c2d-.5
#   c4d2=c2t^2 s4t=s2*c2t c4t=c4d2-.125 s8t=s4t*c4t c8d=c4t^2
#   mixQ: c2s4=c2t*s4t s2c4=s2*c4t   mixK: c2c4=c2t*c4t s2s4=s2*s4t
FIT_W = 0.36
PAIRS = [  # (q_feature, k_feature)
    ("s1", "c1d"), ("c1d", "s1"), ("s2", "c2t"), ("c2t", "s2"),
    ("s4t", "c4t"), ("c4t", "s4t"), ("s8t", "c8d"), ("c8d", "s8t"),
    ("c2s4", "c2c4"), ("s2c4", "s2s4"),
]
ALPHAS = [-0.439680893, -4.32525681, -0.845301755, 0.0239388354,
          -5.14401459, -7.71941257, -175.582672, -319.866805,
          -10.9101526, 2.72429164]
BETAS = [0.0, 1.1382438, 0.0, 0.195903978, 0.0, 0.0444884607,
         0.0, 2.46151355, 0.0, 0.0]


def _install_axon_profile_hook():
    """antenv.axon_hooks is missing from this image; concourse needs it for
    trace=True under axon. Register the ctypes-based NTFF hook manually."""
    import antenv

    if "antenv.axon_hooks" in sys.modules:
        return
    m = types.ModuleType("antenv.axon_hooks")
    m._hook = None

    def _set(h):
        m._hook = h

    def _get():
        return m._hook

    m.set_axon_ntff_profile_hook = _set
    m.get_axon_ntff_profile_hook = _get
    sys.modules["antenv.axon_hooks"] = m
    antenv.axon_hooks = m
    try:
        from trn_agent_boot.trn_boot import _ntff_profile_via_ctypes

        m.set_axon_ntff_profile_hook(
            _ntff_profile_via_ctypes("/opt/axon/libaxon_pjrt.so")
        )
    except Exception:
        pass


def _patch_tile_drain():
    """The walrus build in this image allows at most ONE sync-wait command
    per instruction; Tile's kernel-tail drain carries every vector-clock
    wait on a single drain. Split them across a chain of drains."""
    import concourse.mybir as mybir
    import concourse.tile as tile
    from concourse.vector_clock import ScopedClock

    if getattr(tile.TileContext, "_drain_patched", False):
        return

    def _drain_and_barrier_chunked(self, tick_clock, wait_clock):
        d0 = self.nc.sync.drain()
        wait_clock.add_sem_waits(d0.ins, ScopedClock({None: tick_clock.global_clock}))
        si = d0.ins.sync_info
        waits = list(si.on_wait) if si is not None else []
        if len(waits) > 1:
            engs = [
                mybir.EngineType.SP,
                mybir.EngineType.DVE,
                mybir.EngineType.Activation,
                mybir.EngineType.PE,
                mybir.EngineType.Pool,
            ]
            d0.ins.sync_info = mybir.SyncInfo(
                on_wait=waits[:1], on_update=list(si.on_update)
            )
            for i in range(1, len(waits)):
                ev = mybir.InstEventSemaphore(
                    name=f"tail-wait-{i}",
                    engine=engs[i % len(engs)],
                    ins=[],
                    outs=[],
                    sync_info=mybir.SyncInfo(on_wait=[waits[i]], on_update=[]),
                )
                self.nc.register_instruction(ev)
                self.nc.cur_bb.bb.add_instruction(ev)

        self.nc.all_engine_barrier()
        assert self.sems is not None
        popped = self.nc._tile_sem_poison_stack.pop()
        assert popped is self._sem_poison
        self.nc.clear_and_free_semaphores(list(self.sems.allocated().values()))
        self.nc.all_engine_barrier()

    tile.TileContext._drain_and_barrier = _drain_and_barrier_chunked
    tile.TileContext._drain_patched = True


def _split_multi_waits(nc):
    """walrus here allows one sync-wait command per instruction; move extra
    waits onto standalone EventSemaphore instructions."""
    import concourse.mybir as mybir

    n = 0
    for fn in nc.m.functions:
        for blk in fn.blocks:
            out = []
            for inst in blk.instructions:
                si = inst.sync_info
                waits = list(si.on_wait) if si is not None else []
                if len(waits) > 1:
                    for k, w in enumerate(waits[:-1]):
                        ev = mybir.InstEventSemaphore(
                            name=f"{inst.name}-xw{k}",
                            engine=inst.engine,
                            ins=[],
                            outs=[],
                            sync_info=mybir.SyncInfo(on_wait=[w], on_update=[]),
                        )
                        out.append(ev)
                        n += 1
                    inst.sync_info = mybir.SyncInfo(
                        on_wait=[waits[-1]], on_update=list(si.on_update)
                    )
                out.append(inst)
            blk.instructions = out
    return n


def _ceil_to(x, m):
    return -(-int(x) // m) * m


# feature slot order in the per-side feature tile (each slot = [K | Q]);
# chosen so every batched pair-product has affine operand APs.
SLOTS = {"sh": 0, "s1": 1, "c1d": 2, "c2d": 3, "c2t": 4, "s2": 5,
         "c4d2": 6, "s4t": 7, "c4t": 8, "s8t": 9, "c8d": 10,
         "m1": 11, "m2": 12}
NSLOT = 13
QSLOT_OF = {"c2s4": "m1", "s2c4": "m2"}
KSLOT_OF = {"c2c4": "m1", "s2s4": "m2"}
# NOTE (measured): concurrent GpSimd elementwise ops and DVE ops slow each
# other ~4x via the shared SBUF port lock — GpSimd gets NO elementwise work.


def _build_program(D, V, H, JW, ROWS, NCH, lns, NFULL, LP, split_waits=True):
    """Uniform SPMD program: one batch per core, ROWS query rows, key
    window JW (padded; mask handles validity)."""
    import contextlib

    import concourse.bass as bass
    import concourse.mybir as mybir
    import concourse.tile as tile

    f32 = mybir.dt.float32
    f16 = mybir.dt.float16
    AF = mybir.ActivationFunctionType

    DC = D // 128
    HC = H // 128
    P = len(PAIRS)
    RC = ROWS // 128
    W = FIT_W
    BW = 258  # QC block width: 256 q-cols + 2 beta cols (even => 4B-aligned
    #           blocks keep the DVE tensor_scalar ops in 4x perf mode)

    nc = bass.Bass("TRN2", target_bir_lowering=False)
    d_wk = nc.declare_dram_parameter("wk", [128, DC * H], f16, isOutput=False)
    d_kT = nc.declare_dram_parameter("kT", [128, DC * JW], f16, isOutput=False)
    d_wq = nc.declare_dram_parameter("wq", [128, DC * H], f16, isOutput=False)
    d_qT = nc.declare_dram_parameter("qT", [128, DC * ROWS], f16, isOutput=False)
    if NFULL:
        d_valsf = nc.declare_dram_parameter("valsf", [128, NFULL * V], f16,
                                            isOutput=False)
    if LP:
        d_vals2 = nc.declare_dram_parameter("vals2", [128, V], f16,
                                            isOutput=False)
    # smalls: maskT (NCH cols) | qcs (P*HC cols) f32, wvb f16
    d_sm32 = nc.declare_dram_parameter("sm32", [128, NCH + P * HC], f32,
                                       isOutput=False)
    d_wvb = nc.declare_dram_parameter("wvb", [128, P * HC], f16, isOutput=False)
    d_out = nc.declare_dram_parameter("out", [ROWS, V], f16, isOutput=True)

    KW = HC * JW
    QW = HC * ROWS
    SW = KW + QW

    with tile.TileContext(nc) as tc:
        ctx = contextlib.ExitStack()
        with ctx:
            const_pool = ctx.enter_context(tc.tile_pool(name="const", bufs=1))
            in_pool = ctx.enter_context(tc.tile_pool(name="in", bufs=1))
            feat_pool = ctx.enter_context(tc.tile_pool(name="feat", bufs=1))
            qc_pool = ctx.enter_context(tc.tile_pool(name="qc", bufs=1))
            soft_pool = ctx.enter_context(tc.tile_pool(name="soft", bufs=1))
            out_pool = ctx.enter_context(tc.tile_pool(name="outp", bufs=1))
            ppsum = ctx.enter_context(tc.tile_pool(name="pp", bufs=1, space="PSUM"))
            scpsum = ctx.enter_context(tc.tile_pool(name="scp", bufs=2, space="PSUM"))
            opsum = ctx.enter_context(tc.tile_pool(name="op", bufs=2, space="PSUM"))
            smpsum = ctx.enter_context(tc.tile_pool(name="smp", bufs=1, space="PSUM"))

            warm = const_pool.tile([1, 2], f32)
            ones_f16 = const_pool.tile([128, 1], f16)
            wtile = const_pool.tile([128, 512], f16, name="wtile")
            sm32_sb = const_pool.tile([128, NCH + P * HC], f32, name="sm32")
            wvb_sb = const_pool.tile([128, P * HC], f16, name="wvb")
            maskT = sm32_sb[:, 0:NCH]
            qcs = sm32_sb[:, NCH:NCH + P * HC]

            wk_sb = in_pool.tile([128, DC * H], f16, name="wk")
            kT_sb = in_pool.tile([128, DC * JW], f16, name="kT")
            wq_sb = in_pool.tile([128, DC * H], f16, name="wq")
            qT_sb = in_pool.tile([128, DC * ROWS], f16, name="qT")
            if NFULL:
                valsf_sb = in_pool.tile([128, NFULL * V], f16, name="valsf")
            if LP:
                vals2_sb = in_pool.tile([128, V], f16, name="vals2")

            F = feat_pool.tile([128, NSLOT * SW], f16, name="F")
            QC = qc_pool.tile([128, P * HC * BW], f16, name="QC")
            eT = soft_pool.tile([128, NCH * ROWS], f16, name="eT")
            biasc = soft_pool.tile([128, NCH], f32, name="biasc")
            rinv = soft_pool.tile([128, RC], f32, name="rinv")
            out_sb = out_pool.tile([128, RC * V], f16, name="osb")

            pk = ppsum.tile([128, HC * 512], f32, tag="pk", name="pk")
            pq = ppsum.tile([128, 512], f32, tag="pq", name="pq")
            misc = smpsum.tile([128, 512], f32, tag="misc", name="misc")

            # ---- memsets (DVE) then DMA triggers, one per tensor
            nc.vector.memset(wtile[:], 0.25)
            nc.vector.memset(warm[:], 0.5)
            nc.vector.memset(ones_f16[:], 1.0)
            # k-projection inputs split across BOTH HWDGE queues so they
            # stream concurrently; q inputs next, values last.
            nc.sync.dma_start(out=wk_sb[:], in_=d_wk[:])
            # kT in two halves so the first k-proj matmuls start earlier
            kh = (DC // 2) * JW
            nc.scalar.dma_start(out=kT_sb[:, 0:kh], in_=d_kT[:, 0:kh])
            nc.scalar.dma_start(out=kT_sb[:, kh:], in_=d_kT[:, kh:])
            nc.sync.dma_start(out=qT_sb[:], in_=d_qT[:])
            # Sin table load rides between the scalar queue's triggers
            nc.scalar.activation(warm[0:1, 0:1], warm[0:1, 1:2], AF.Sin)
            nc.scalar.dma_start(out=wq_sb[:], in_=d_wq[:])
            if NFULL:
                nc.scalar.dma_start(out=valsf_sb[:], in_=d_valsf[:])
            if LP:
                nc.sync.dma_start(out=vals2_sb[:LP, :], in_=d_vals2[:LP, :])
            nc.gpsimd.dma_start(out=sm32_sb[:], in_=d_sm32[:])
            nc.gpsimd.dma_start(out=wvb_sb[:], in_=d_wvb[:])

            # beta columns -> cols 256,257 of each QC block (the pad col gets
            # the same value so nothing in the block is uninitialized; psum
            # col 257 is simply never read)
            qcv = QC[:]
            wvb_src = wvb_sb[:, 0:P * HC].rearrange("p (b c) -> p b c", c=1)
            for pad in (256, 257):
                wvb_dst = bass.AP(qcv.tensor, qcv.offset + pad,
                                  [qcv.ap[0], [BW, P * HC], [1, 1]])
                nc.vector.tensor_copy(wvb_dst, wvb_src)

            # ---- PE warm-up: keep PE busy through the DMA so HAM un-throttles
            for _ in range(NWARM):
                nc.tensor.matmul(pq[:, 0:512], wtile[:, 0:128],
                                 wtile[:, 0:512], start=True, stop=True)

            # ---- projections (PE); pk h-chunks bank-aligned at hc*512
            for hc in range(HC):
                for dc in range(DC):
                    nc.tensor.matmul(
                        pk[:, hc * 512:hc * 512 + JW],
                        wk_sb[:, dc * H + hc * 128:dc * H + (hc + 1) * 128],
                        kT_sb[:, dc * JW:(dc + 1) * JW],
                        start=(dc == 0), stop=(dc == DC - 1),
                    )
            for hc in range(HC):
                for dc in range(DC):
                    nc.tensor.matmul(
                        pq[:, hc * ROWS:(hc + 1) * ROWS],
                        wq_sb[:, dc * H + hc * 128:dc * H + (hc + 1) * 128],
                        qT_sb[:, dc * ROWS:(dc + 1) * ROWS],
                        start=(dc == 0), stop=(dc == DC - 1),
                    )

            # ---- feature slot helpers (slot s = [K (KW) | Q (QW)])
            def kslot(s):
                o = SLOTS[s] * SW
                return F[:, o:o + KW]

            def qslot(s):
                o = SLOTS[s] * SW + KW
                return F[:, o:o + QW]

            # ---- sines: ONE ACT instr per side per pass
            pkap = pk[:]
            pk_src = bass.AP(pkap.tensor, pkap.offset,
                             [pkap.ap[0], [512, HC], [1, JW]])
            # all K sines first: the K chain (score stationaries) is the
            # longest dependency spine, so unblock it earliest
            for scale, s in ((0.5 * W, "sh"), (W, "s1"), (2.0 * W, "s2")):
                nc.scalar.activation(
                    kslot(s).rearrange("p (c j) -> p c j", j=JW),
                    pk_src, AF.Sin, scale=scale)
            for scale, s in ((0.5 * W, "sh"), (W, "s1"), (2.0 * W, "s2")):
                nc.scalar.activation(qslot(s), pq[:, 0:QW], AF.Sin, scale=scale)
            # second PE warm-up block: covers the sines/chain window (PE
            # would otherwise idle >3.4us and HAM would re-throttle); targets
            # the misc bank, which has NO readers yet, so it runs dep-free
            for _ in range(8):
                nc.tensor.matmul(misc[:, 0:512], wtile[:, 0:128],
                                 wtile[:, 0:512], start=True, stop=True)

            # ---- DVE product chains, K and Q sides fused per instruction
            fb = F[:]

            def kp_(sa, sb):
                return bass.AP(fb.tensor, fb.offset + SLOTS[sa] * SW,
                               [fb.ap[0], [(SLOTS[sb] - SLOTS[sa]) * SW, 2],
                                [1, KW]])

            def qp_(sa, sb):
                return bass.AP(fb.tensor, fb.offset + SLOTS[sa] * SW + KW,
                               [fb.ap[0], [(SLOTS[sb] - SLOTS[sa]) * SW, 2],
                                [1, QW]])

            def qc_block(p, hc):
                blk = p * HC + hc
                return QC[:, blk * BW:blk * BW + 256]

            def qc_scale(p, act=False):
                qs2 = QSLOT_OF.get(PAIRS[p][0], PAIRS[p][0])
                for hc in range(HC):
                    o = SLOTS[qs2] * SW + KW + hc * ROWS
                    if act:
                        nc.scalar.activation(
                            qc_block(p, hc), F[:, o:o + ROWS], AF.Copy,
                            scale=qcs[:, p * HC + hc:p * HC + hc + 1])
                    else:
                        nc.vector.tensor_scalar_mul(
                            qc_block(p, hc), F[:, o:o + ROWS],
                            qcs[:, p * HC + hc:p * HC + hc + 1])

            # Everything elementwise on DVE (GpSimd would poison it via the
            # shared SBUF port); pairs 6/7 QC ride ACT's idle window.
            # K chain first (its features are the score stationaries):
            nc.vector.tensor_mul(kp_("c1d", "c2d"), kp_("sh", "s1"),
                                 kp_("sh", "s1"))
            nc.vector.tensor_scalar_sub(kslot("c2t"), kslot("c2d"), 0.5)
            nc.vector.tensor_mul(kp_("c4d2", "s4t"), kp_("c2t", "c2t"),
                                 kp_("c2t", "s2"))
            nc.vector.tensor_scalar_sub(kslot("c4t"), kslot("c4d2"), 0.125)
            nc.vector.tensor_mul(kp_("s8t", "c8d"), kp_("s4t", "c4t"),
                                 kp_("c4t", "c4t"))
            # early-pair QC rides ACT (idle after the sines; p0/p2 read
            # ACT's own sine outputs); DVE keeps the chain spine + late QC
            qc_scale(0, act=True)   # q-feat s1
            qc_scale(2, act=True)   # q-feat s2
            # both K mix slots in ONE fused op: (m1,m2) = (c2t,s2)*(c4t,s4t);
            # the second source pair steps backwards (c4t@8 -> s4t@7)
            nc.vector.tensor_mul(kp_("m1", "m2"), kp_("c2t", "s2"),
                                 kp_("c4t", "s4t"))
            # K feature slots complete here.
            # Q chain with per-pair QC as each feature lands:
            nc.vector.tensor_mul(qp_("c1d", "c2d"), qp_("sh", "s1"),
                                 qp_("sh", "s1"))
            qc_scale(1, act=True)
            nc.vector.tensor_scalar_sub(qslot("c2t"), qslot("c2d"), 0.5)
            qc_scale(3, act=True)
            nc.vector.tensor_mul(qp_("c4d2", "s4t"), qp_("c2t", "c2t"),
                                 qp_("c2t", "s2"))
            qc_scale(4, act=True)
            # Exp table load after ACT's QC block (data-dep blocks hoisting)
            nc.scalar.activation(warm[0:1, 0:1], qslot("s2")[0:1, 0:1], AF.Exp)
            nc.vector.tensor_scalar_sub(qslot("c4t"), qslot("c4d2"), 0.125)
            qc_scale(5)
            nc.vector.tensor_mul(qp_("s8t", "c8d"), qp_("s4t", "c4t"),
                                 qp_("c4t", "c4t"))
            qc_scale(6)
            nc.vector.tensor_mul(qp_("m1", "m2"), qp_("c2t", "s2"),
                                 qp_("s4t", "c4t"))
            qc_scale(7)
            qc_scale(8)
            qc_scale(9)

            # ---- scores scT[j,i], PAIR-major: each QC block arrival unlocks
            # all NCH jc-chunks' matmuls at once, so PE stays busy (and the
            # HAM clock gate stays open) while the QC blocks trickle in.
            pouts = [opsum.tile([128, V], f32, tag="po", name=f"pout{rc}")
                     for rc in range(RC)]

            def vals_view(jc, ln):
                if jc < NFULL:
                    return valsf_sb[:ln, jc * V:(jc + 1) * V]
                return vals2_sb[:ln, :]

            def score_mm(psc, p, jc, hc, start, stop):
                ln = lns[jc]
                ks = KSLOT_OF.get(PAIRS[p][1], PAIRS[p][1])
                nc.tensor.matmul(
                    psc[:ln, 0:BW],
                    F[:, SLOTS[ks] * SW + hc * JW + jc * 128:
                       SLOTS[ks] * SW + hc * JW + jc * 128 + ln],
                    QC[:, (p * HC + hc) * BW:(p * HC + hc + 1) * BW],
                    start=start, stop=stop,
                )

            def emit_softmax(jc):
                ln = lns[jc]
                psc = pscs[jc]
                nc.vector.tensor_add(biasc[:ln, jc:jc + 1],
                                     maskT[:ln, jc:jc + 1],
                                     psc[:ln, 256:257])
                nc.scalar.activation(
                    eT[:ln, jc * ROWS:(jc + 1) * ROWS],
                    psc[:ln, 0:ROWS], AF.Exp,
                    bias=biasc[:ln, jc:jc + 1],
                )

            def emit_values(jc):
                ln = lns[jc]
                for rc in range(RC):
                    eblk = eT[:ln, jc * ROWS + rc * 128:jc * ROWS + rc * 128 + 128]
                    # groups interleave across jc, so each accumulator needs
                    # its own PSUM bank: rc0 sums -> misc, rc1 sums -> pq;
                    # pout rc0 -> opsum, pout rc1 -> a pk bank (dead now)
                    rsum_dst = misc[:, 0:1] if rc == 0 else pq[:, 0:1]
                    pout_dst = pouts[rc][:]
                    nc.tensor.matmul(
                        rsum_dst, eblk, ones_f16[:ln, 0:1],
                        start=(jc == 0), stop=(jc == NCH - 1))
                    nc.tensor.matmul(
                        pout_dst, eblk, vals_view(jc, ln),
                        start=(jc == 0), stop=(jc == NCH - 1))

            # jc-major: consecutive matmuls accumulate into the SAME psum
            # bank, which keeps the PE back-to-back pipeline (pair-major
            # bank-alternation measured ~3x slower per matmul)
            pscs = []
            for jc in range(NCH):
                psc = scpsum.tile([128, 512], f32, tag="sc",
                                  name=f"psc{jc}")
                pscs.append(psc)
                nmm = 0
                for p in range(P):
                    for hc in range(HC):
                        score_mm(psc, p, jc, hc, start=(nmm == 0),
                                 stop=(nmm == P * HC - 1))
                        nmm += 1
                    if jc == 0 and 1 <= p <= 6:
                        # dep-free filler matmuls: the first score group is
                        # paced by the QC trickle (~30% PE duty), which lets
                        # the HAM clock gate re-throttle; keep the PE busy.
                        # Pairs 1-3 wait on ACT's slower QC cadence and
                        # showed residual ~1us gaps -> one extra filler.
                        for _ in range(3 if p <= 3 else 2):
                            nc.tensor.matmul(misc[:, 0:256], wtile[:, 0:128],
                                             wtile[:, 0:256],
                                             start=True, stop=True)
                emit_softmax(jc)
                if jc > 0:
                    emit_values(jc - 1)
            emit_values(NCH - 1)

            # ---- normalize + store
            nc.vector.reciprocal(rinv[:, 0:1], misc[:, 0:1])
            nc.vector.reciprocal(rinv[:, 1:2], pq[:, 0:1])
            for rc in range(RC):
                pout_src = pouts[rc][:]
                # split the two output scales across ACT and DVE
                if rc == 0:
                    nc.scalar.activation(
                        out_sb[:, rc * V:(rc + 1) * V], pout_src,
                        AF.Copy, scale=rinv[:, rc:rc + 1])
                else:
                    nc.vector.tensor_scalar_mul(
                        out_sb[:, rc * V:(rc + 1) * V], pout_src,
                        rinv[:, rc:rc + 1])
                nc.sync.dma_start(
                    out=d_out[rc * 128:(rc + 1) * 128, :],
                    in_=out_sb[:, rc * V:(rc + 1) * V])

    if split_waits:
        _split_multi_waits(nc)
    return nc


def _pack_pm(a, cols):
    """[D, cols] -> partition-major [128, (D//128)*cols] fp16."""
    D = a.shape[0]
    dc = D // 128
    return np.ascontiguousarray(
        a.reshape(dc, 128, cols).transpose(1, 0, 2).reshape(128, dc * cols)
        .astype(np.float16))


def kernel(queries, keys, values, valid_lens, W_q, W_k, w_v):
    global LAST_RESULT
    _install_axon_profile_hook()
    _patch_tile_drain()
    from concourse.bass_utils import run_bass_kernel_spmd

    f16 = np.float16
    queries = np.ascontiguousarray(queries, dtype=np.float32)
    keys = np.ascontiguousarray(keys, dtype=np.float32)
    values = np.ascontiguousarray(values, dtype=np.float32)
    W_q = np.ascontiguousarray(W_q, dtype=np.float32)
    W_k = np.ascontiguousarray(W_k, dtype=np.float32)
    w_v = np.ascontiguousarray(w_v, dtype=np.float32)
    vl = np.asarray(valid_lens).astype(np.int64)

    B, Q, D = queries.shape
    KV = keys.shape[1]
    V = values.shape[2]
    H = W_q.shape[1]
    P = len(PAIRS)
    CPB = NCORES // B          # cores per batch
    ROWS = Q // CPB            # query rows per core
    HC = H // 128

    jms = [min(KV, int(v)) for v in vl]
    JW = min(KV, _ceil_to(max(jms), 8))
    NCH = _ceil_to(JW, 128) // 128
    lns = [min(128, JW - jc * 128) for jc in range(NCH)]
    LP = 0 if JW % 128 == 0 else JW - (NCH - 1) * 128
    NFULL = NCH if LP == 0 else NCH - 1

    # fold sign(w_v) into the projections (tanh is odd)
    sgn = np.where(w_v >= 0, 1.0, -1.0).astype(np.float32)
    wva = np.abs(w_v)
    Wq_f = _pack_pm(W_q * sgn[None, :], H)
    Wk_f = _pack_pm(W_k * sgn[None, :], H)

    nc = _build_program(D, V, H, JW, ROWS, NCH, lns, NFULL, LP)

    qcs_cols = [ALPHAS[p] * wva[hc * 128:(hc + 1) * 128]
                for p in range(P) for hc in range(HC)]
    wvb_cols = [BETAS[p] * wva[hc * 128:(hc + 1) * 128]
                for p in range(P) for hc in range(HC)]
    qcs = np.stack(qcs_cols, axis=1).astype(np.float32)
    wvb = np.ascontiguousarray(np.stack(wvb_cols, axis=1).astype(f16))

    in_maps = []
    for c in range(NCORES):
        b = c // CPB
        rh = c % CPB
        jm = jms[b]
        qT = _pack_pm(
            np.ascontiguousarray(queries[b, rh * ROWS:(rh + 1) * ROWS, :].T),
            ROWS)
        kTp = np.zeros((D, JW), np.float32)
        kTp[:, :jm] = keys[b, :jm, :].T
        kT = _pack_pm(kTp, JW)
        j = np.arange(128)
        maskT = np.stack(
            [np.where(jc * 128 + j < jm, 0.0, NEG).astype(np.float32)
             for jc in range(NCH)], axis=1)
        sm32 = np.ascontiguousarray(
            np.concatenate([maskT, qcs], axis=1).astype(np.float32))
        im = {"wk": Wk_f, "kT": kT, "wq": Wq_f, "qT": qT,
              "sm32": sm32, "wvb": wvb}
        if NFULL:
            vf = np.zeros((128, NFULL * V), f16)
            for jc in range(NFULL):
                nrows = max(0, min(128, jm - jc * 128))
                if nrows:
                    vf[0:nrows, jc * V:(jc + 1) * V] = \
                        values[b, jc * 128:jc * 128 + nrows, :].astype(f16)
            im["valsf"] = np.ascontiguousarray(vf)
        if LP:
            v2 = np.zeros((128, V), f16)
            nrows = max(0, min(LP, jm - NFULL * 128))
            if nrows:
                v2[0:nrows] = values[b, NFULL * 128:NFULL * 128 + nrows, :] \
                    .astype(f16)
            im["vals2"] = v2
        in_maps.append(im)

    res = run_bass_kernel_spmd(
        nc, in_maps, core_ids=list(range(NCORES)), trace=TRACE
    )
    LAST_RESULT = res

    out = np.empty((B, Q, V), np.float32)
    for c in range(NCORES):
        b = c // CPB
        rh = c % CPB
        out[b, rh * ROWS:(rh + 1) * ROWS, :] = res.results[c]["out"].astype(
            np.float32)
    return out
